# revision 10
# baseline (speedup 1.0000x reference)
"""RNN-T JointNetwork kernel for Trainium2 (Bass/Tile), SPMD over 8 NeuronCores.

Computes, per batch element b (one per core):
    h_enc = x_enc[b] @ w_l + b_l          # (T, H)
    h_prd = x_prd[b] @ w_p + b_p          # (U, H)
    h     = tanh(h_enc[t] + h_prd[u])     # (T, U, H)
    out   = h @ w_h + b_h                 # (T, U, V)

The graded metric is wall-clock of kernel() in a 1-CPU container with
axon-tunneled devices (~90MB/s, ~0.5s/transfer fixed), so the design
minimizes wire bytes and host passes, not just device time:
  * All large inputs ship as ONE packed bf16 tensor per core (x_enc,
    x_prd, w_l, w_p, w_h); biases ship as one small f32 tensor.
  * GEMMs run in bf16 (full-rate PE, fp32 PSUM accumulation).
  * Output ships int8 row-quantized: per output row r the device also
    emits scale[r] = absmax(row)/127; the host dequantizes in a single
    fused ufunc pass (cast+mul+write).  rel err ~ bf16 (3.4e-3) +
    1/254 (2e-3 avg) << 2e-2 gate.  QUANT_INT8=False falls back to a
    plain fp16 output.

Device layout (per core) is unchanged from the f32 baseline:
  * h kept feature-major (h on partitions) so h tiles feed the big GEMM
    as the stationary operand directly; rows ordered r' = u*T + t so the
    broadcast-add + tanh is ONE scalar-engine op per (u, H-tile).
  * Big GEMM: V split into two 512-wide PSUM banks, 4 k-tiles of H.
  * Output rows r' map to logits rows r = t*U + u; each 128-row tile
    stores with <=2 DMAs per V-half.  Scales are stored in r' order
    (contiguous, 1 DMA/tile); the host permutes them (U*T f32, tiny).
"""

import sys

for _p in ("/opt/trn_rl_repo",):
    if _p not in sys.path:
        sys.path.insert(0, _p)

import numpy as np

B, T, U = 8, 200, 50
E = H = 512
V = 1024
P = 128
KT = E // P  # 4 contraction tiles for the small GEMMs
HT = H // P  # 4 contraction tiles for the big GEMM
R = T * U    # rows per core
N_CORES = 8
CHUNKS = [2, 16, 16, 16]  # u-chunks; first small to fill the pipeline fast
QUANT_INT8 = True

# packed bf16 input: element offsets
OFF_XE = 0
OFF_XP = OFF_XE + T * E
OFF_WL = OFF_XP + U * E
OFF_WP = OFF_WL + E * H
OFF_WH = OFF_WP + E * H
NPACK = OFF_WH + H * V
NBIAS = H + H + V  # b_l | b_p | b_h, f32

_CACHE = {}
_last_in_maps = None


def _emit(nc, tc, tile, mybir):
    f32 = mybir.dt.float32
    bf16 = mybir.dt.bfloat16
    f16 = mybir.dt.float16
    i8 = mybir.dt.int8
    Act = mybir.ActivationFunctionType
    Alu = mybir.AluOpType

    packed_d = nc.dram_tensor("packed", [NPACK], bf16, kind="ExternalInput")
    bias_d = nc.dram_tensor("biases", [NBIAS], f32, kind="ExternalInput")
    if QUANT_INT8:
        out_d = nc.dram_tensor("out", [R, V], i8, kind="ExternalOutput")
        scl_d = nc.dram_tensor("scl", [R], f32, kind="ExternalOutput")
    else:
        out_d = nc.dram_tensor("out", [R, V], f16, kind="ExternalOutput")

    x_enc_d = packed_d[OFF_XE:OFF_XE + T * E].rearrange("(t e) -> t e", e=E)
    x_prd_d = packed_d[OFF_XP:OFF_XP + U * E].rearrange("(u e) -> u e", e=E)
    w_l_d = packed_d[OFF_WL:OFF_WL + E * H].rearrange("(e h) -> e h", h=H)
    w_p_d = packed_d[OFF_WP:OFF_WP + E * H].rearrange("(e h) -> e h", h=H)
    w_h_d = packed_d[OFF_WH:OFF_WH + H * V].rearrange("(h v) -> h v", v=V)
    b_l_d = bias_d[0:H]
    b_p_d = bias_d[H:2 * H]
    b_h_d = bias_d[2 * H:2 * H + V]

    from concourse.masks import make_identity
    from contextlib import ExitStack

    ctx = ExitStack()
    cpool = ctx.enter_context(tc.tile_pool(name="const", bufs=1))
    pbig = ctx.enter_context(tc.tile_pool(name="pbig", bufs=4, space="PSUM"))
    hcpool = ctx.enter_context(tc.tile_pool(name="hc", bufs=2))
    opool = ctx.enter_context(tc.tile_pool(name="op", bufs=6))

    ident = cpool.tile([P, P], bf16, tag="ident")
    make_identity(nc, ident[:])

    # ---- inputs that gate the PE pipeline come first ----
    xe_nat = []
    t_sizes = []
    t0 = 0
    while t0 < T:
        ti = min(P, T - t0)
        t_ = cpool.tile([P, E], bf16, tag=f"xen{len(xe_nat)}",
                        name=f"xen{len(xe_nat)}")
        nc.sync.dma_start(out=t_[:ti, :], in_=x_enc_d[t0:t0 + ti, :])
        xe_nat.append(t_)
        t_sizes.append(ti)
        t0 += ti
    xp_nat = cpool.tile([P, E], bf16, tag="xpn")
    nc.sync.dma_start(out=xp_nat[:U, :], in_=x_prd_d[:, :])

    wl = []
    for k in range(KT):
        t_ = cpool.tile([P, H], bf16, tag=f"wl{k}", name=f"wl{k}")
        nc.sync.dma_start(out=t_[:], in_=w_l_d[k * P:(k + 1) * P, :])
        wl.append(t_)
    bl = cpool.tile([P, KT], f32, tag="bl")
    nc.sync.dma_start(out=bl[:], in_=b_l_d.rearrange("(a p) -> p a", p=P))
    wp = []
    for k in range(KT):
        t_ = cpool.tile([P, H], bf16, tag=f"wp{k}", name=f"wp{k}")
        nc.sync.dma_start(out=t_[:], in_=w_p_d[k * P:(k + 1) * P, :])
        wp.append(t_)
    bp = cpool.tile([P, KT], f32, tag="bp")
    nc.sync.dma_start(out=bp[:], in_=b_p_d.rearrange("(a p) -> p a", p=P))

    # ---- transpose x_enc / x_prd on the PE (feature dim -> partitions) ----
    xeT = [cpool.tile([P, T], bf16, tag=f"xeT{k}", name=f"xeT{k}")
           for k in range(KT)]
    xpT = [cpool.tile([P, U], bf16, tag=f"xpT{k}", name=f"xpT{k}")
           for k in range(KT)]
    _rr = [0]
    def _pstile(shape, dt=f32):
        _rr[0] ^= 1
        return pbig.tile(shape, dt, tag=f"ps{_rr[0]}", name="pss")

    for k in range(KT):
        t0 = 0
        for i, ti in enumerate(t_sizes):
            ps = _pstile([P, 512], bf16)
            nc.tensor.transpose(
                ps[:, :ti], xe_nat[i][:ti, k * P:(k + 1) * P], ident[:ti, :ti]
            )
            nc.scalar.copy(xeT[k][:, t0:t0 + ti], ps[:, :ti])
            t0 += ti
        ps = _pstile([P, 512], bf16)
        nc.tensor.transpose(
            ps[:, :U], xp_nat[:U, k * P:(k + 1) * P], ident[:U, :U]
        )
        nc.scalar.copy(xpT[k][:, :U], ps[:, :U])

    # ---- small GEMMs: h_encT [H, T], h_prdT [H, U] (+bias via ACT) ----
    heT = [cpool.tile([P, T], f32, tag=f"heT{j}", name=f"heT{j}")
           for j in range(HT)]
    hpT = [cpool.tile([P, U], f32, tag=f"hpT{j}", name=f"hpT{j}")
           for j in range(HT)]
    for j in range(HT):
        ps = _pstile([P, 512])
        for k in range(KT):
            nc.tensor.matmul(
                ps[:, :T],
                wl[k][:, j * P:(j + 1) * P],
                xeT[k][:, :T],
                start=(k == 0),
                stop=(k == KT - 1),
            )
        nc.scalar.activation(
            heT[j][:], ps[:, :T], Act.Identity, bias=bl[:, j:j + 1]
        )
    for j in range(HT):
        ps = _pstile([P, 512])
        for k in range(KT):
            nc.tensor.matmul(
                ps[:, :U],
                wp[k][:, j * P:(j + 1) * P],
                xpT[k][:, :U],
                start=(k == 0),
                stop=(k == KT - 1),
            )
        nc.scalar.activation(
            hpT[j][:], ps[:, :U], Act.Identity, bias=bp[:, j:j + 1]
        )

    # ---- big-GEMM weights last: not needed until the first chunk's GEMM ----
    wh = []
    for k in range(HT):
        t_ = cpool.tile([P, V], bf16, tag=f"wh{k}", name=f"wh{k}")
        nc.sync.dma_start(out=t_[:], in_=w_h_d[k * P:(k + 1) * P, :])
        wh.append(t_)
    bh_rep = cpool.tile([P, V], f32, tag="bh")
    nc.sync.dma_start(
        out=bh_rep[:], in_=b_h_d.unsqueeze(0).broadcast_to([P, V])
    )

    # ---- main loop over u-chunks; rows r' = u*T + t ----
    out_view = out_d[:].rearrange("(t u) v -> u t v", u=U)
    max_cu = max(CHUNKS)
    u0 = 0
    for cu in CHUNKS:
        rc = cu * T
        hc = [hcpool.tile([P, max_cu * T], bf16, tag=f"hc{j}", name=f"hc{j}")
              for j in range(HT)]
        # fused broadcast-add + tanh; du-outer so early GEMM tiles unblock
        for du in range(cu):
            for j in range(HT):
                nc.scalar.activation(
                    hc[j][:, du * T:(du + 1) * T],
                    heT[j][:, :T],
                    Act.Tanh,
                    bias=hpT[j][:, u0 + du:u0 + du + 1],
                )
        # big GEMM over 128-row tiles of this chunk
        for m0 in range(0, rc, P):
            m = min(P, rc - m0)
            g0 = u0 * T + m0  # global r' row of this tile
            ps0 = pbig.tile([P, 512], f32, tag="ps0")
            ps1 = pbig.tile([P, 512], f32, tag="ps1")
            for j in range(HT):
                lhsT = hc[j][:, m0:m0 + m]
                nc.tensor.matmul(
                    ps0[:m, :], lhsT, wh[j][:, 0:512],
                    start=(j == 0), stop=(j == HT - 1),
                )
                nc.tensor.matmul(
                    ps1[:m, :], lhsT, wh[j][:, 512:V],
                    start=(j == 0), stop=(j == HT - 1),
                )
            if QUANT_INT8:
                # bias-add + per-row absmax in one DVE pass per V-half
                ot0 = opool.tile([P, 512], f32, tag="ot0", name="ot0")
                ot1 = opool.tile([P, 512], f32, tag="ot1", name="ot1")
                ra0 = opool.tile([P, 1], f32, tag="ra0", bufs=3)
                ra1 = opool.tile([P, 1], f32, tag="ra1", bufs=3)
                inv = opool.tile([P, 1], f32, tag="inv", bufs=3)
                qs = opool.tile([P, 1], f32, tag="qs", bufs=3)
                st = opool.tile([P, 1], f32, tag="st", bufs=3)
                q0 = opool.tile([P, 512], i8, tag="q0", name="q0")
                q1 = opool.tile([P, 512], i8, tag="q1", name="q1")
                nc.vector.tensor_add(ot0[:m], ps0[:m], bh_rep[:m, 0:512])
                nc.vector.tensor_add(ot1[:m], ps1[:m], bh_rep[:m, 512:V])
                nc.vector.tensor_reduce(
                    out=ra0[:m], in_=ot0[:m], axis=mybir.AxisListType.X,
                    op=Alu.max, apply_absolute_value=True,
                )
                nc.vector.tensor_reduce(
                    out=ra1[:m], in_=ot1[:m], axis=mybir.AxisListType.X,
                    op=Alu.max, apply_absolute_value=True,
                )
                nc.vector.tensor_max(ra1[:m], ra0[:m], ra1[:m])
                nc.vector.tensor_scalar_max(ra1[:m], ra1[:m], 1e-12)
                nc.vector.reciprocal(inv[:m], ra1[:m])
                nc.vector.tensor_scalar_mul(qs[:m], inv[:m], 127.0)
                nc.vector.tensor_scalar_mul(st[:m], ra1[:m], 1.0 / 127.0)
                # quantize on the scalar engine (DVE stays on the reduces)
                nc.scalar.mul(q0[:m], ot0[:m], qs[:m])
                nc.scalar.mul(q1[:m], ot1[:m], qs[:m])
                nc.sync.dma_start(out=scl_d[g0:g0 + m].unsqueeze(1),
                                  in_=st[:m])
                for v, qv in ((0, q0), (1, q1)):
                    seg = m0
                    while seg < m0 + m:
                        du = seg // T
                        tA = seg % T
                        seg_len = min(m0 + m, (du + 1) * T) - seg
                        nc.sync.dma_start(
                            out=out_view[
                                u0 + du, tA:tA + seg_len, v * 512:(v + 1) * 512
                            ],
                            in_=qv[seg - m0:seg - m0 + seg_len, :],
                        )
                        seg += seg_len
            else:
                # epilogue per V-half so each PSUM bank drains + stores
                # independently; store rows split at u boundaries (<=2 segs)
                for v, psv in ((0, ps0), (1, ps1)):
                    ot = opool.tile([P, 512], f16, tag=f"ot{v}", name=f"ot{v}")
                    nc.vector.tensor_add(
                        ot[:m, :], psv[:m, :], bh_rep[:m, v * 512:(v + 1) * 512]
                    )
                    seg = m0
                    while seg < m0 + m:
                        du = seg // T
                        tA = seg % T
                        seg_len = min(m0 + m, (du + 1) * T) - seg
                        nc.sync.dma_start(
                            out=out_view[
                                u0 + du, tA:tA + seg_len, v * 512:(v + 1) * 512
                            ],
                            in_=ot[seg - m0:seg - m0 + seg_len, :],
                        )
                        seg += seg_len
        u0 += cu

    ctx.close()


def _build():
    if "nc" in _CACHE:
        return _CACHE["nc"]
    from concourse import bacc, mybir
    import concourse.tile as tile

    nc = bacc.Bacc("TRN2", target_bir_lowering=False, debug=False)
    with tile.TileContext(nc) as tc:
        _emit(nc, tc, tile, mybir)
    nc.compile()
    _CACHE["nc"] = nc
    return nc


def kernel(**inputs):
    import ml_dtypes
    from concourse.bass_utils import run_bass_kernel_spmd

    bf16 = ml_dtypes.bfloat16
    nc = _build()
    x_enc = np.asarray(inputs["x_enc"], dtype=np.float32).astype(bf16)
    x_prd = np.asarray(inputs["x_prd"], dtype=np.float32).astype(bf16)
    w_flat = np.concatenate([
        np.asarray(inputs["w_l"], np.float32).astype(bf16).ravel(),
        np.asarray(inputs["w_p"], np.float32).astype(bf16).ravel(),
        np.asarray(inputs["w_h"], np.float32).astype(bf16).ravel(),
    ])
    biases = np.concatenate([
        np.asarray(inputs["b_l"], np.float32).ravel(),
        np.asarray(inputs["b_p"], np.float32).ravel(),
        np.asarray(inputs["b_h"], np.float32).ravel(),
    ])
    in_maps = []
    for b in range(N_CORES):
        packed = np.concatenate([
            x_enc[b, :, 0, :].ravel(),
            x_prd[b, 0, :, :].ravel(),
            w_flat,
        ])
        in_maps.append({"packed": packed, "biases": biases})

    global _last_in_maps
    _last_in_maps = in_maps
    res = run_bass_kernel_spmd(nc, in_maps, core_ids=list(range(N_CORES)))

    out = np.empty((N_CORES, T, U, V), np.float32)
    if QUANT_INT8:
        for b in range(N_CORES):
            q = res.results[b]["out"]            # (R, V) int8, r = t*U+u
            s = res.results[b]["scl"]            # (R,) f32,  r' = u*T+t
            s_r = np.ascontiguousarray(s.reshape(U, T).T).reshape(R, 1)
            np.multiply(q, s_r, out=out[b].reshape(R, V), dtype=np.float32)
    else:
        for b in range(N_CORES):
            o = res.results[b]["out"]            # (R, V) fp16
            out[b] = o.reshape(T, U, V).astype(np.float32)
    return out
